# revision 12
# baseline (speedup 1.0000x reference)
"""Trainium2 Bass kernel for degree-3 uniform B-spline basis evaluation.

Problem: x (1024, 8192) fp32, knots = linspace(-2, 2, 12) -> out (1024, 8192, 8);
out[..., i] is the i-th cubic B-spline basis function (Cox-de Boor).

Math. With uniform knots (spacing h), out_i(x) = C(u - i) for the cardinal
cubic C. Writing a = |(x-k0)/h - i - 2|, k2 = (2/3)^(1/3), s = 4^(1/3), and
the single clamped precursor r = relu(2*k2 - a*k2):

    4*C = r^3 - relu(s*r - s*k2)^2 * (s*r - s*k2)

(one-input identity: on the clamped region r=0 the second term's argument is
-s*k2 < 0, so both terms vanish exactly; verified to 3e-15 against Cox-de
Boor). The stored value is 4*C, a power-of-two multiple, so the host's *0.25
up-cast is exact.

Engine split per span of F columns (real-HW rates: ACT 0.65 ns/elem, DVE
custom ~1.1 ns/elem on planar contiguous APs; Pool tensor ops and DVE
strided-bf16 writes are software-slow and avoided entirely):

  ACT   q2_i = Abs(x*(k2/h) - c_i*(k2/h))    8 ops    -> q2 [P,8,F] fp32
        r2 = Relu(2k2 - q2)                  1 wide op -> r2 [P,8,F] fp32
  DVE   out4 = r2^3 - relu(s*r2 - sk2)^2*(s*r2 - sk2)
        ONE 8-ALU custom instruction over [P,8,F], planar bf16 out
  DMA   outp [P,8,F] bf16 -> out_dram [P, 8, COLS] channel-planar
        (per-partition 8 runs of 2KB at COLS stride)

The channel-planar bf16 output is up-cast host-side (contiguous astype,
exact *0.25) and returned as a zero-copy channel-last transposed view.

Sharding: batch-parallel, rows 128*c .. 128*c+127 on core c (8 cores).
"""

import numpy as np

_CACHE = {}

_K2 = float((2.0 / 3.0) ** (1.0 / 3.0))  # k2^3 = 2/3
_S = float(4.0 ** (1.0 / 3.0))           # s^3 = 4

_P = 128
_COLS = 8192
_NB = 8
_F = 1024
_NCORES = 8


def _register_custom_ops():
    import concourse.dve_ops as dve_ops
    from concourse.dve_ops import DveOp
    from concourse.dve_spec import Spec, Src0, C0, C1, relu, sq, lower
    from concourse.dve_uop import DveOpSpec

    def _reg(name, body, ref):
        ex = {op.name: op for op in dve_ops.OPS}
        if name in ex:
            return ex[name]
        spec = Spec(body=body, reference=ref)
        shas = {v: DveOpSpec(name=name, uops=lower(spec, ver=v)).sha(v)
                for v in ("v3", "v4")}
        op = DveOp(name, spec, subdim=False, uops_sha=shas)
        dve_ops.OPS.append(op)
        dve_ops.CUSTOM_DVE_SPECS[name] = op.spec
        row = max(dve_ops._SUB_OPCODE_FOR_NAME.values()) + 1
        assert row < 0x20
        dve_ops._SUB_OPCODE_FOR_NAME[name] = row
        return op

    from concourse.dve_spec import AluOp, Bin

    # q2 = |x - c| * (k2/h): the per-channel Abs, for channels offloaded
    # from ACT to DVE's slack.
    def _ref_abs(in0, in1, s0, s1, imm2):
        return (np.abs(in0.astype(np.float32) - s0) * s1).astype(np.float32)

    def _body_abs():
        return Bin(AluOp.ABSOLUTE_DIFF, Src0, C0) * C1

    # in0 = r2 = relu(2k2 - a*k2) >= 0 (pre-clamped on ACT), C0 = s,
    # C1 = s*k2.  out = r2^3 - relu(s*r2 - s*k2)^2 * (s*r2 - s*k2) = 4*C.
    def _ref(in0, in1, s0, s1, imm2):
        r2 = in0.astype(np.float32)
        p = np.square(r2) * r2
        w = r2 * s0 - s1
        return (p - np.square(np.maximum(w, 0)) * w).astype(np.float32)

    def _body():
        p = sq(Src0) * Src0
        w = Src0 * C0 - C1
        return p - sq(relu(w)) * w

    return (_reg("BSPL_G", _body(), _ref),
            _reg("BSPL_ABS", _body_abs(), _ref_abs))


def _build(knot0: float, h: float, passes: int = 1):
    import concourse.bacc as bacc
    import concourse.mybir as mybir
    from concourse import tile

    AF = mybir.ActivationFunctionType
    bspl_g, bspl_abs = _register_custom_ops()

    nc = bacc.Bacc("TRN2", target_bir_lowering=False, debug=False,
                   num_devices=_NCORES)
    x_ext = nc.declare_dram_parameter("x", [_P, _COLS], mybir.dt.float32,
                                      isOutput=False)
    out_ext = nc.declare_dram_parameter("out", [_P, _NB, _COLS],
                                        mybir.dt.bfloat16, isOutput=True)

    with tile.TileContext(nc) as tc:
        with tc.tile_pool(name="xin", bufs=3) as xin, \
             tc.tile_pool(name="q2p", bufs=1) as q2p, \
             tc.tile_pool(name="r2p", bufs=3) as r2p, \
             tc.tile_pool(name="outp", bufs=3) as outp, \
             tc.tile_pool(name="cst", bufs=1) as cst:
            cvals = sorted({-(knot0 + (i + 2) * h) * _K2 / h
                            for i in range(_NB)} | {2.0 * _K2})
            for v in cvals:
                t = cst.tile([_P, 1], mybir.dt.float32, tag=f"c{v}")
                nc.vector.memset(t[:], float(v))
                nc.const_aps.aps[(mybir.dt.float32, float(v))] = t
            nspan = _COLS // _F

            ALU = mybir.AluOpType
            _ND = 2  # channels whose |x - c| runs on DVE tensor_scalar

            def _prologue(s):
                xs = xin.tile([_P, _F], mybir.dt.float32, tag="x")
                nc.sync.dma_start(xs[:], x_ext[:, s * _F:(s + 1) * _F])
                q2 = q2p.tile([_P, _NB, _F], mybir.dt.float32, tag="q2")
                for i in range(_NB - _ND):
                    c_i = knot0 + (i + 2) * h
                    nc.scalar.activation(q2[:, i, :], xs[:], AF.Abs,
                                         bias=-c_i * _K2 / h, scale=_K2 / h)
                for j in range(_ND):
                    c_i = knot0 + (_NB - _ND + j + 2) * h
                    nc.vector._custom_dve(
                        bspl_abs, out=q2[:, _NB - _ND + j, :], in0=xs[:],
                        s0=c_i, s1=_K2 / h)
                r2 = r2p.tile([_P, _NB, _F], mybir.dt.float32, tag="r2")
                nc.scalar.activation(r2[:], q2[:], AF.Relu,
                                     bias=2.0 * _K2, scale=-1.0)
                return r2

            for rep in range(passes):
                # Two-deep software pipeline: spans s+1 and s+2's ACT
                # prologues are in flight while span s's DVE/store runs.
                pend = [_prologue(0), _prologue(1)]
                for s in range(nspan):
                    r2 = pend.pop(0)
                    if s + 2 < nspan:
                        pend.append(_prologue(s + 2))
                    o4 = outp.tile([_P, _NB, _F], mybir.dt.bfloat16, tag="o4")
                    nc.vector._custom_dve(
                        bspl_g, out=o4[:], in0=r2[:],
                        s0=_S, s1=_S * _K2)
                    nc.sync.dma_start(out_ext[:, :, s * _F:(s + 1) * _F],
                                      o4[:])

    nc.compile()
    return nc


def _numpy_fallback(x, knots):
    te = x[..., None]
    B = ((knots[:-1] <= te) & (te < knots[1:])).astype(np.float32)
    nk = len(knots)
    for k in range(1, 4):
        n = nk - k - 1
        ld = knots[k:k + n] - knots[:n]
        rd = knots[k + 1:k + 1 + n] - knots[1:1 + n]
        left = np.where(ld != 0, (te - knots[:n]) / ld, 0.0) * B[..., :n]
        right = (np.where(rd != 0, (knots[k + 1:k + 1 + n] - te) / rd, 0.0)
                 * B[..., 1:n + 1])
        B = (left + right).astype(np.float32)
    return B[..., :_NB]


def kernel(x: np.ndarray, knots: np.ndarray | None = None, **_ignored):
    from concourse.bass_utils import run_bass_kernel_spmd

    x = np.ascontiguousarray(np.asarray(x, dtype=np.float32))
    if knots is None:
        knots = np.linspace(-2.0, 2.0, 12, dtype=np.float32)
    knots = np.asarray(knots, dtype=np.float32)
    assert x.shape == (_P * _NCORES, _COLS), x.shape
    knot0 = float(knots[0])
    h = float(knots[-1] - knots[0]) / (len(knots) - 1)
    if not np.allclose(np.diff(knots), h, rtol=1e-5, atol=1e-6):
        return _numpy_fallback(x, knots)

    key = (knot0, h)
    if key not in _CACHE:
        _CACHE[key] = _build(knot0, h)
    nc = _CACHE[key]

    in_maps = [{"x": x[c * _P:(c + 1) * _P]} for c in range(_NCORES)]
    res = run_bass_kernel_spmd(nc, in_maps, list(range(_NCORES)))
    planar = np.empty((_P * _NCORES, _NB, _COLS), dtype=np.float32)
    for c in range(_NCORES):
        planar[c * _P:(c + 1) * _P] = res.results[c]["out"].astype(np.float32)
    planar *= 0.25  # exact: stored value is 4*C, a power-of-two multiple
    return planar.transpose(0, 2, 1)
